# revision 1
# baseline (speedup 1.0000x reference)
"""Trainium2 Bass kernel for nn_BasicBlock1D (locally-connected 1x1 conv x2
with training-mode BatchNorm, residual, ReLU).

Reference computation (per spatial position h, there are H=64 of them):
    out1[n,o,h] = sum_c x[n,c,h] * w1[o,c,h]          (512x512 matmul per h)
    y1 = relu(bn1(out1))                              (stats over (N,H))
    out2[n,o,h] = sum_c y1[n,c,h] * w2[o,c,h]
    y  = relu(bn2(out2) + x)

Sharding: the 64 spatial positions are split across the 8 NeuronCores (8 per
core).  Each core reads only its h-slice of x/w1/w2, so every HBM byte is
read exactly once chip-wide.  BatchNorm statistics span the full (N,H) batch,
so each core computes per-channel partial moments and a tiny (6KB) AllGather
(+local reduce) combines them; everything else is core-local.

Layouts are pre-packed on the host so all device DMAs are large and
contiguous, with the channel (contraction) axis on SBUF partitions:
    x  -> [kc, p, h, n]   (c = kc*128 + p)
    w  -> [h2, p, hh, kc, o]  pairs of h per tile
    out <- [oc, p, h, n]
Matmuls run in bf16 (fp32 PSUM accumulate); BN statistics and all
normalization math are fp32.

Structure per h-pair: each PSUM tile is a full bank [128, 2, 256] holding
both h of the pair for one output chunk; 8 matmuls accumulate into it and a
single op evacuates it (split 2 chunks on ACT / 2 on DVE so neither engine
backpressures the PE).  bn_stats per (chunk, h-pair) runs as soon as its
data lands, so the layer-end stats tail is one h-pair deep, not layer-deep.

The final phase uses relu(s2*(out2 + x/s2) + t2) = relu(s2*out2 + t2 + x):
GpSimd does x/s2 and the residual add, ACT fuses the affine + relu.

DMA queues: weights + outputs on the SP HWDGE ring; x and small constants on
the ACT HWDGE ring (runs in parallel); collective bounce buffers on GpSimd.
A dummy Sqrt at t=0 preloads the ACT function table off the critical path.

BN moment bookkeeping: bn_stats on a 512-element group yields
(cnt,mean,M2) for even/odd element substreams (count 256 each).  With
A = sum of all substream means, B = sum of all M2, C = sum of squared
substream means (summed over groups, then AllGather-reduced over cores):
    mean = A/64,   E[x^2] = B/16384 + C/64,   var = E[x^2] - mean^2
(64 substreams of 256 elements = 16384 samples).

Stack quirks this kernel deliberately avoids (verified empirically on this
axon/PJRT toolchain): tensor_tensor_reduce (faults), tensor_tensor with the
same tile as both operands, DVE memset feeding scalar operands, float
immediates in tensor_scalar, in-place elementwise ops, and instructions
whose only output has no reader (walrus drops the alloc and the engine
faults).
"""

import os
import sys
from contextlib import ExitStack

import numpy as np

_REPO = "/opt/trn_rl_repo"
if _REPO not in sys.path:
    sys.path.insert(0, _REPO)

import ml_dtypes  # noqa: E402

import concourse.bacc as bacc  # noqa: E402
import concourse.tile as tile  # noqa: E402
from concourse import mybir  # noqa: E402
from concourse.bass_utils import run_bass_kernel_spmd  # noqa: E402

N, C, H = 256, 512, 64
NCORES = 8
HS = H // NCORES  # 8 h positions per core
P = 128
KC = C // P  # 4 contraction chunks
OC = C // P  # 4 output-channel chunks
NN = N  # moving free dim of each matmul
HPAIRS = HS // 2  # weight tiles / activation tiles hold 2 h positions
NSUB = 2 * HPAIRS * NCORES  # substreams per channel globally = 64
M_GLOBAL = float(N * H)  # BatchNorm population count = 16384
EPS = 1e-5

BF16 = mybir.dt.bfloat16
F32 = mybir.dt.float32

LAST_EXEC_NS = None
LAST_RESULTS = None

_cached = None


def _build_program():
    nc = bacc.Bacc(
        "TRN2",
        target_bir_lowering=False,
        debug=False,
        num_devices=NCORES,
    )

    xt_d = nc.dram_tensor("xt", [KC, P, HS, NN], BF16, kind="ExternalInput")
    w1_d = nc.dram_tensor("w1t", [HPAIRS, P, 2, KC, C], BF16, kind="ExternalInput")
    w2_d = nc.dram_tensor("w2t", [HPAIRS, P, 2, KC, C], BF16, kind="ExternalInput")
    g1_d = nc.dram_tensor("g1t", [P, OC], F32, kind="ExternalInput")
    b1_d = nc.dram_tensor("b1t", [P, OC], F32, kind="ExternalInput")
    g2_d = nc.dram_tensor("g2t", [P, OC], F32, kind="ExternalInput")
    b2_d = nc.dram_tensor("b2t", [P, OC], F32, kind="ExternalInput")
    cst_d = nc.dram_tensor("cst", [P, 4], F32, kind="ExternalInput")
    out_d = nc.dram_tensor("out", [OC, P, HS, NN], F32, kind="ExternalOutput")
    junk_d = nc.dram_tensor("junk", [P, 1], F32, kind="ExternalOutput")

    add = mybir.AluOpType.add
    mult = mybir.AluOpType.mult
    AF = mybir.ActivationFunctionType
    use_cc = os.environ.get("KERNEL_NOCC", "0") != "1"

    with tile.TileContext(nc) as tc, ExitStack() as ctx:
        persist = ctx.enter_context(tc.tile_pool(name="persist", bufs=1))
        wpool = ctx.enter_context(tc.tile_pool(name="wpool", bufs=6))
        spool = ctx.enter_context(tc.tile_pool(name="spool", bufs=2))
        psum = ctx.enter_context(tc.tile_pool(name="psum", bufs=8, space="PSUM"))
        dram = ctx.enter_context(tc.tile_pool(name="dram", bufs=1, space="DRAM"))

        def hp_tiles(nm, dt, n_hp=HPAIRS):
            return [
                [
                    persist.tile([P, 2, NN], dt, tag=f"{nm}_{k}_{hp}", name=f"{nm}_{k}_{hp}")
                    for hp in range(n_hp)
                ]
                + [None] * (HPAIRS - n_hp)
                for k in range(OC)
            ]

        # --- persistent activations ---
        xs = [persist.tile([P, HS, NN], BF16, tag=f"x{k}", name=f"x{k}") for k in range(KC)]
        raw1 = hp_tiles("r1", BF16)   # layer-1 pre-BN output
        y1 = hp_tiles("y1", BF16)
        o2 = hp_tiles("o2", F32)      # layer-2 pre-BN output

        # ACT function-table preload: a dummy Sqrt as the very first ACT op
        # pulls in the (sqrt + basics) table off the critical path.
        dummy_in = persist.tile([P, 1], F32, tag="dmy_i", name="dmy_i")
        nc.vector.memset(dummy_in, 4.0)
        dummy_out = persist.tile([P, 1], F32, tag="dmy_o", name="dmy_o")
        nc.scalar.activation(out=dummy_out, in_=dummy_in, func=AF.Sqrt)
        nc.scalar.dma_start(out=junk_d.ap(), in_=dummy_out)

        # x input first on the ACT HWDGE ring; the weight stream starts in
        # parallel on the SP ring.  Small constant DMAs are emitted later so
        # they don't delay anything on the critical path.
        for k in range(KC):
            nc.scalar.dma_start(out=xs[k], in_=xt_d.ap()[k])

        gbs = {}
        cst = persist.tile([P, 4], F32, tag="cst", name="cst")
        eps_ap = cst[:, 0:1]
        inv_m_ap = cst[:, 1:2]  # 1/16384
        inv_s_ap = cst[:, 2:3]  # 1/64

        def load_small_consts():
            for nm, d in (("g1", g1_d), ("b1", b1_d), ("g2", g2_d), ("b2", b2_d)):
                t = persist.tile([P, OC], F32, tag=nm, name=f"gb_{nm}")
                nc.scalar.dma_start(out=t, in_=d.ap())
                gbs[nm] = t
            nc.scalar.dma_start(out=cst, in_=cst_d.ap())

        def layer(w_d, src_at, dst_tiles, lname):
            """Per-position matmuls + per-channel partial BN moments.

            src_at(kc, h) -> [P, NN] AP of the layer input
            dst_tiles[oc][hp][:, hh, :] <- the (h = 2*hp+hh) output slice
            returns stats tile [P, 3*OC]: (A, B, C) per output chunk.
            """
            st_raw = persist.tile(
                [P, OC, HPAIRS, 6], F32, tag=f"straw_{lname}", name=f"straw_{lname}"
            )
            for hp in range(HPAIRS):
                w = wpool.tile([P, 2, KC, C], BF16, tag="w", name="w")
                weng = nc.sync if hp % 2 == 0 else nc.scalar
                weng.dma_start(out=w, in_=w_d.ap()[hp])
                for oc in range(OC):
                    # full-bank PSUM tile: both h of the pair
                    pt = psum.tile([P, 2, NN], F32, tag="ps", name="ps")
                    for hh in range(2):
                        h = hp * 2 + hh
                        for kc in range(KC):
                            nc.tensor.matmul(
                                pt[:, hh, :],
                                lhsT=w[:, hh, kc, oc * P : (oc + 1) * P],
                                rhs=src_at(kc, h),
                                start=(kc == 0),
                                stop=(kc == KC - 1),
                            )
                    # single-op PSUM evacuation on ACT (DVE owns bn_stats)
                    nc.scalar.activation(
                        out=dst_tiles[oc][hp], in_=pt, func=AF.Copy
                    )
                # BN partial moments per h-pair as soon as its data lands
                # (for the resident pair, straight from PSUM)
                for oc in range(OC):
                    nc.vector.bn_stats(
                        out=st_raw[:, oc, hp, :],
                        in_=dst_tiles[oc][hp].rearrange("p a n -> p (a n)"),
                    )
            return st_raw

        def gather_stats(st_raw, lname):
            """AllGather the raw bn_stats 6-tuples; one bn_aggr per chunk
            then yields global (mean, var) directly."""
            W = OC * HPAIRS * 6
            mv = persist.tile([P, OC, 2], F32, tag=f"mv_{lname}", name=f"mv_{lname}")
            if use_cc:
                cc_in = dram.tile([P, W], F32, tag=f"cci_{lname}", name=f"cci_{lname}")
                cc_out = dram.tile(
                    [NCORES, P, W], F32, tag=f"cco_{lname}", name=f"cco_{lname}"
                )
                nc.gpsimd.dma_start(
                    out=cc_in, in_=st_raw.rearrange("p a b c -> p (a b c)")
                )
                nc.gpsimd.collective_compute(
                    "AllGather",
                    mybir.AluOpType.bypass,
                    replica_groups=[list(range(NCORES))],
                    ins=[cc_in.opt()],
                    outs=[cc_out.opt()],
                )
                gath = persist.tile(
                    [P, NCORES, OC, HPAIRS, 6], F32,
                    tag=f"gth_{lname}", name=f"gth_{lname}",
                )
                nc.gpsimd.dma_start(
                    out=gath,
                    in_=cc_out.rearrange("r p w -> p r w").rearrange(
                        "p r (a b c) -> p r a b c", a=OC, b=HPAIRS
                    ),
                )
                for oc in range(OC):
                    nc.vector.bn_aggr(out=mv[:, oc, :], in_=gath[:, :, oc, :, :])
            else:
                for oc in range(OC):
                    nc.vector.bn_aggr(out=mv[:, oc, :], in_=st_raw[:, oc, :, :])
            return mv

        def bn_coeffs(mv, g_t, b_t, lname):
            """scale/shift so that bn(v) = s*v + t, per channel.

            mv[:, oc, 0] = mean, mv[:, oc, 1] = var.  No in-place ops, no
            same-tile-twice operands (stack quirks).
            """

            def small(nm):
                return persist.tile([P, OC], F32, tag=f"{nm}_{lname}", name=f"{nm}_{lname}")

            std = small("std")
            rstd = small("rstd")
            s_t = small("s")
            mts = small("mts")
            t_t = small("t")
            nc.scalar.activation(
                out=std, in_=mv[:, :, 1], func=AF.Sqrt, bias=eps_ap, scale=1.0
            )
            nc.vector.reciprocal(out=rstd, in_=std)
            nc.vector.tensor_mul(out=s_t, in0=rstd, in1=g_t)
            nc.vector.tensor_mul(out=mts, in0=mv[:, :, 0], in1=s_t)
            nc.vector.tensor_sub(out=t_t, in0=b_t, in1=mts)
            return s_t, t_t

        # ---------------- layer 1 ----------------
        stats1 = layer(w1_d, lambda kc, h: xs[kc][:, h, :], raw1, "l1")
        load_small_consts()
        tot1 = gather_stats(stats1, "l1")
        s1, t1 = bn_coeffs(tot1, gbs["g1"], gbs["b1"], "l1")
        # y1 = relu(s1*out1 + t1), per (h-pair, chunk); hp-outer order so
        # the first layer-2 matmul group unblocks after 4 applies
        for hp in range(HPAIRS):
            for oc in range(OC):
                if (hp * OC + oc) % 2 == 0:
                    nc.scalar.activation(
                        out=y1[oc][hp],
                        in_=raw1[oc][hp],
                        func=AF.Relu,
                        scale=s1[:, oc : oc + 1],
                        bias=t1[:, oc : oc + 1],
                    )
                else:
                    ytmp = spool.tile([P, 2, NN], F32, tag="ya", name="ya", bufs=3)
                    nc.vector.tensor_scalar(
                        out=ytmp,
                        in0=raw1[oc][hp],
                        scalar1=s1[:, oc : oc + 1],
                        scalar2=t1[:, oc : oc + 1],
                        op0=mult,
                        op1=add,
                    )
                    nc.vector.tensor_relu(out=y1[oc][hp], in_=ytmp)

        # ---------------- layer 2 ----------------
        stats2 = layer(w2_d, lambda kc, h: y1[kc][h // 2][:, h % 2, :], o2, "l2")
        tot2 = gather_stats(stats2, "l2")
        s2, t2 = bn_coeffs(tot2, gbs["g2"], gbs["b2"], "l2")
        # y = relu((s2*out2 + t2) + x): per h-pair, the affine and the
        # residual add alternate between DVE and GpSimd; all relus on ACT
        # (which is otherwise idle in the tail), store per chunk.
        for oc in range(OC):
            f2 = spool.tile([P, HS, NN], F32, tag="f2", name="f2")
            outb = spool.tile([P, HS, NN], F32, tag="outb", name="outb")
            for hp in range(HPAIRS):
                sl = slice(2 * hp, 2 * hp + 2)
                f1 = spool.tile([P, 2, NN], F32, tag="f1", name="f1", bufs=4)
                e1 = nc.gpsimd if (oc * HPAIRS + hp) % 2 == 0 else nc.vector
                nc.vector.tensor_scalar(
                    out=f1,
                    in0=o2[oc][hp],
                    scalar1=s2[:, oc : oc + 1],
                    scalar2=t2[:, oc : oc + 1],
                    op0=mult,
                    op1=add,
                )
                e1.tensor_tensor(
                    out=f2[:, sl, :], in0=f1, in1=xs[oc][:, sl, :], op=add
                )
                if (oc + hp) % 2 == 0:
                    nc.vector.tensor_relu(out=outb[:, sl, :], in_=f2[:, sl, :])
                else:
                    nc.scalar.activation(
                        out=outb[:, sl, :], in_=f2[:, sl, :], func=AF.Relu
                    )
            nc.sync.dma_start(out=out_d.ap()[oc], in_=outb)

    nc.compile()
    return nc


def _get_program():
    global _cached
    if _cached is None:
        _cached = _build_program()
    return _cached


def _pack_inputs(x, w1, g1, b1, w2, g2, b2):
    """Host-side shard + repack into the device layouts (see module doc)."""
    bf16 = ml_dtypes.bfloat16
    # x: (N, C, H) -> [kc, p, h, n]
    xt = np.ascontiguousarray(x.transpose(1, 2, 0)).reshape(KC, P, H, N)
    xt = xt.astype(bf16)

    # w: (O, C, H) -> [h, p, kc, o] -> grouped in h-pairs [h2, p, 2, kc, o]
    def packw(w):
        wt = w.transpose(2, 1, 0).reshape(H, KC, P, C).transpose(0, 2, 1, 3)
        return wt.astype(bf16)  # (H, P, KC, C)

    w1t = packw(w1)
    w2t = packw(w2)

    def packg(v):
        return np.ascontiguousarray(v.reshape(OC, P).T.astype(np.float32))

    g1t, b1t, g2t, b2t = packg(g1), packg(b1), packg(g2), packg(b2)
    cst = np.empty((P, 4), np.float32)
    cst[:, 0] = EPS
    cst[:, 1] = 1.0 / M_GLOBAL
    cst[:, 2] = 1.0 / float(NSUB)
    cst[:, 3] = 0.0

    in_maps = []
    for c in range(NCORES):
        h0, h1 = c * HS, (c + 1) * HS
        in_maps.append(
            {
                "xt": np.ascontiguousarray(xt[:, :, h0:h1, :]),
                "w1t": np.ascontiguousarray(w1t[h0:h1]).reshape(
                    HPAIRS, 2, P, KC, C
                ).transpose(0, 2, 1, 3, 4).copy(),
                "w2t": np.ascontiguousarray(w2t[h0:h1]).reshape(
                    HPAIRS, 2, P, KC, C
                ).transpose(0, 2, 1, 3, 4).copy(),
                "g1t": g1t,
                "b1t": b1t,
                "g2t": g2t,
                "b2t": b2t,
                "cst": cst,
            }
        )
    return in_maps


def kernel(x, w1, g1, b1, w2, g2, b2):
    global LAST_EXEC_NS, LAST_RESULTS
    nc = _get_program()
    in_maps = _pack_inputs(
        np.asarray(x, dtype=np.float32),
        np.asarray(w1, dtype=np.float32),
        np.asarray(g1, dtype=np.float32),
        np.asarray(b1, dtype=np.float32),
        np.asarray(w2, dtype=np.float32),
        np.asarray(g2, dtype=np.float32),
        np.asarray(b2, dtype=np.float32),
    )
    trace = os.environ.get("KERNEL_TRACE", "0") == "1"
    res = run_bass_kernel_spmd(
        nc, in_maps, list(range(NCORES)), trace=trace
    )
    LAST_EXEC_NS = res.exec_time_ns
    LAST_RESULTS = res
    parts = []
    for c in range(NCORES):
        r = np.asarray(res.results[c]["out"])  # [oc, p, h, n]
        parts.append(r.reshape(C, HS, N).transpose(2, 0, 1))  # (n, c, h)
    return np.concatenate(parts, axis=2).astype(np.float32)


if __name__ == "__main__":
    # smoke test with random data
    rng = np.random.default_rng(0)
    x = rng.standard_normal((N, C, H), dtype=np.float32)
    w1 = rng.standard_normal((C, C, H), dtype=np.float32) * 0.02
    w2 = rng.standard_normal((C, C, H), dtype=np.float32) * 0.02
    g1 = np.ones(C, np.float32)
    b1 = np.zeros(C, np.float32)
    g2 = np.ones(C, np.float32)
    b2 = np.zeros(C, np.float32)
    y = kernel(x=x, w1=w1, g1=g1, b1=b1, w2=w2, g2=g2, b2=b2)
    print(y.shape, y.dtype, float(np.abs(y).max()))

